# revision 26
# baseline (speedup 1.0000x reference)
"""Trainium2 Bass kernel for GAT-style single-query attention.

Reference computation (N=16384, D=1024, H=8):
    scores[n,h] = leaky_relu(x0 @ Wi[h] + x[n] @ Wj[h] + b[h], 0.01)
    probs       = softmax(scores, axis=n)  (per head)
    out[d]      = relu(mean_h(sum_n probs[n,h] * x[n,d]))

Strategy (v3): shard rows (N) across 8 cores.  Each core:
  - one packed DMA for the small inputs (W|b|x0 concatenated host-side),
    then 8 X-group DMAs (2 k-chunks of 128 rows each) issued immediately,
  - each X group is cast f32 -> bf16 once (DVE / GpSimd alternating);
    every PE instruction downstream runs in bf16 (1 cycle/row -- measured
    f32r streams at ~2 cycles/row on HW, so bf16 halves matmul time),
  - X^T via PE transposes of the bf16 tiles into 1-bank PSUM tiles
    ([128,1024] bf16), one 2x-speed DVE copy per c-pair,
  - scores^T [8, <=512] per supergroup; the per-head constant
    cvec[h] = x0 @ Wi[h] + b[h] is folded into the PSUM->SBUF copy as a
    per-partition tensor_scalar add (h is the partition dim there),
  - scores^T transposed back to natural [128, k, 8] layout on the PE
    (tiny 8-row transposes), so exp/leaky runs on few-free-element tiles
    (DVE/ACT cost ~ free size, partitions are parallel),
  - u = exp(leaky(s)) = max(exp(s), exp(0.01 s)) (exp monotone; scores
    are in [-9, 8] for this distribution, no max-subtraction needed),
  - HO[h, d] += u^T X on the PE (u natural stationary, X natural moving),
  - supergroups sized [4,4,4,2,2] k-chunks so the post-DMA tail is short,
  - ships HO partials [8, 1024] plus the raw u tile [128, 128] (f32);
    the host reduces Z_h and finishes relu(mean_h HO_h / Z_h).
"""

import sys

sys.path.insert(0, "/opt/trn_rl_repo")

import numpy as np

import concourse.bacc as bacc
import concourse.tile as tile
from concourse import mybir
from concourse import masks
from concourse.bass_utils import run_bass_kernel_spmd

N, D, H = 16384, 1024, 8
NCORES = 8
NSHARD = N // NCORES          # 2048 rows per core
KCH = NSHARD // 128           # 16 n-chunks of 128 rows
DCH = 8                       # d-chunks of 128 cols
NDMA = 8                      # DMA groups (2 k-chunks each)
KPD = KCH // NDMA             # k-chunks per DMA group (2)
SGS = [(0, 4), (4, 8), (8, 12), (12, 14), (14, 16)]  # supergroup k-ranges
F32 = mybir.dt.float32
BF16 = mybir.dt.bfloat16
WBX_W = 2 * D + 1 + 128       # W[8,2048] | b[8,1] | x0 as [8,128]
N_WARMUP = 3                  # lean f32 warm-up matmuls


def _build():
    nc = bacc.Bacc("TRN2", target_bir_lowering=False, debug=False,
                   num_devices=NCORES)
    x_in = nc.dram_tensor("x", [NSHARD, D], F32, kind="ExternalInput").ap()
    wbx_in = nc.dram_tensor("wbx", [H, WBX_W], F32,
                            kind="ExternalInput").ap()
    ho_out = nc.dram_tensor("ho", [H, D], F32, kind="ExternalOutput").ap()
    z_out = nc.dram_tensor("z", [128, KCH * H], F32,
                           kind="ExternalOutput").ap()

    with tile.TileContext(nc) as tc:
        with (
            tc.tile_pool(name="consts", bufs=1) as consts,
            tc.tile_pool(name="small", bufs=1) as small,
            tc.tile_pool(name="xn", bufs=1) as xn_pool,
            tc.tile_pool(name="xb", bufs=1) as xb_pool,
            tc.tile_pool(name="xt", bufs=1) as xt_pool,
            tc.tile_pool(name="st", bufs=2) as st_pool,
            tc.tile_pool(name="ee", bufs=2) as e_pool,
            tc.tile_pool(name="pt", bufs=4, space="PSUM") as pt_pool,
            tc.tile_pool(name="ps", bufs=1, space="PSUM") as ps_pool,
            tc.tile_pool(name="pho", bufs=1, space="PSUM") as pho_pool,
            tc.tile_pool(name="pu", bufs=1, space="PSUM") as pu_pool,
        ):
            # ---- warm-up operands: the very first DVE instructions ----
            wa = consts.tile([128, H], F32)
            nc.vector.memset(wa[:], 0.001)
            wb = consts.tile([128, 512], F32)
            nc.vector.memset(wb[:], 0.001)

            # identity on gpsimd (parallel with the DVE memsets)
            id128 = consts.tile([128, 128], F32)
            masks.make_identity(nc, id128[:])
            id128b = consts.tile([128, 128], BF16)
            nc.vector.tensor_copy(id128b[:], id128[:])

            # preload the ACT exp table during the DMA wait
            act_dummy = consts.tile([1, H], F32)
            nc.scalar.activation(act_dummy[:], wa[0:1, :],
                                 mybir.ActivationFunctionType.Exp)

            # ---- DMA: X group 0 first, then the small input, then the
            # remaining X groups (queues drain in trigger order) ----
            wbx_sb = small.tile([H, WBX_W], F32)
            x_view = x_in.rearrange("(p k) d -> p k d", k=KCH)
            xn_tiles = [xn_pool.tile([128, KPD, D], F32, tag=f"xn{g}",
                                     name=f"xn{g}")
                        for g in range(NDMA)]
            nc.sync.dma_start(out=wbx_sb[:], in_=wbx_in[:])
            for g in range(NDMA):
                nc.sync.dma_start(
                    out=xn_tiles[g][:],
                    in_=x_view[:, g * KPD:(g + 1) * KPD, :])

            # ---- PE warm-up: 3 f32 matmuls on alternating banks ----
            ho0 = pho_pool.tile([H, 512], F32, tag="ho0")
            ho1 = pho_pool.tile([H, 512], F32, tag="ho1")
            for i in range(N_WARMUP):
                nc.tensor.matmul((ho0 if i % 2 == 0 else ho1)[:],
                                 wa[:], wb[:], start=True, stop=True)

            # bf16 copy of the packed small input (for W/x0 transposes)
            wbx_b = small.tile([H, WBX_W], BF16)
            nc.vector.tensor_copy(wbx_b[:], wbx_sb[:])


            # bf16 casts of the X groups (cast engine interleaved so the
            # PE never waits: first group of each sg on DVE early, second
            # on ACT).  sg0's casts go out NOW -- before the W copies,
            # which are gated on PE transposes and would delay them.
            xb_tiles = {}

            def emit_cast(g, eng):
                # split each group cast DVE/ACT half-and-half: the two
                # halves run in parallel, halving the cast latency on the
                # DMA->cast->transpose critical chain
                if g in xb_tiles:
                    return
                xb = xb_pool.tile([128, KPD, D], BF16, tag=f"xb{g}")
                xb_tiles[g] = xb
                nc.vector.tensor_copy(xb[:, 0, :], xn_tiles[g][:, 0, :])
                nc.scalar.copy(xb[:, 1, :], xn_tiles[g][:, 1, :])

            def sg_groups(si):
                k0, k1 = SGS[si]
                return sorted({k // KPD for k in range(k0, k1)})

            gs0 = sg_groups(0)
            emit_cast(gs0[0], nc.vector)
            if len(gs0) > 1:
                emit_cast(gs0[1], nc.scalar)

            # sg0 casts early in DVE/ACT program order (W copies below
            # are gated on PE transposes and would delay them by ~3us)

            # ---- W^T chunks via bf16 PE transposes ----
            wt_j = small.tile([128, DCH, H], BF16)
            wt_i = small.tile([128, DCH, H], BF16)
            for half, dst in ((1, wt_j), (0, wt_i)):
                pw = pt_pool.tile([128, 2 * 512], BF16, tag="pt")
                for c in range(DCH):
                    nc.tensor.matmul(
                        pw[:, c * H:(c + 1) * H],
                        wbx_b[:, (half * DCH + c) * 128:
                              (half * DCH + c + 1) * 128],
                        id128b[:H, :H],
                        is_transpose=True, start=True, stop=True)
                nc.vector.tensor_copy(dst[:], pw[:, 0:DCH * H])

            # x0^T: [8, 128] slice -> [128, 8]
            px = pt_pool.tile([128, 2 * 512], BF16, tag="pt")
            nc.tensor.matmul(px[:, 0:H], wbx_b[:, 2 * D + 1:],
                             id128b[:H, :H],
                             is_transpose=True, start=True, stop=True)
            x0t = small.tile([128, H], BF16)
            nc.vector.tensor_copy(x0t[:], px[:, 0:H])

            # cvec[h] = x0 @ Wi[h] + b[h], as an [8, 1] column
            pc = ps_pool.tile([64, 512], F32, tag="ps")
            for c in range(DCH):
                nc.tensor.matmul(pc[0:H, 0:1], wt_i[:, c, :],
                                 x0t[:, c:c + 1],
                                 start=(c == 0), stop=(c == DCH - 1))
            cvec_c = small.tile([H, 1], F32)
            nc.vector.tensor_tensor(cvec_c[:], pc[0:H, 0:1],
                                    wbx_sb[:, 2 * D:2 * D + 1],
                                    mybir.AluOpType.add)

            # u partial sums, natural layout: [128, k, 8] across all k
            u_all = small.tile([128, KCH, H], BF16)


            def emit_xp_half(si, state, which):
                """Transposes for supergroup si, group-half `which`.
                PSUM tiles are per (c-quad, half) so each tile completes
                within one phase and its copy can run mid-supergroup."""
                k0, k1 = SGS[si]
                gs = sg_groups(si)
                if which == 1 and len(gs) == 1:
                    return
                gsel = gs[which]
                js = [j for j in range(k1 - k0)
                      if (k0 + j) // KPD == gsel]
                for P in range(2):
                    ptt = pt_pool.tile([128, 1024], BF16, tag="pt",
                                       name=f"ptt{si}_{P}_{which}")
                    state[(P, which)] = ptt
                    for ci in range(4):
                        c = P * 4 + ci
                        for jj, j in enumerate(js):
                            k = k0 + j
                            g, j2 = k // KPD, k % KPD
                            nc.tensor.matmul(
                                ptt[:, ci * 256 + jj * 128:
                                    ci * 256 + (jj + 1) * 128],
                                xb_tiles[g][:, j2, c * 128:(c + 1) * 128],
                                id128b[:],
                                is_transpose=True, start=True, stop=True)

            def emit_copies_half(si, state, which, xts):
                k0, k1 = SGS[si]
                gs = sg_groups(si)
                if which == 1 and len(gs) == 1:
                    return xts
                nj = len([j for j in range(k1 - k0)
                          if (k0 + j) // KPD == gs[which]])
                if xts is None:
                    xts = [xt_pool.tile([128, 4, 512], BF16,
                                        tag=f"xt{si}_{P}",
                                        name=f"xt{si}_{P}")
                           for P in range(2)]
                for P in range(2):
                    ptt = state[(P, which)]
                    ptt3 = ptt[:].rearrange("p (a b) -> p a b", b=256)
                    nc.vector.tensor_copy(
                        xts[P][:, :, which * 256:which * 256 + nj * 128],
                        ptt3[:, :, 0:nj * 128])
                return xts

            # prologue: transposes + copies for sg 0 (casts already out)
            state0 = {}
            emit_xp_half(0, state0, 0)
            xts_cur = emit_copies_half(0, state0, 0, None)
            emit_xp_half(0, state0, 1)
            xts_cur = emit_copies_half(0, state0, 1, xts_cur)
            state_next = None
            xts_next = None
            for si, (k0, k1) in enumerate(SGS):
                nk = k1 - k0
                ncol = nk * 128
                # scores^T [8, ncol] for this supergroup
                ps_t = ps_pool.tile([64, 512], F32, tag="ps")
                for c in range(DCH):
                    nc.tensor.matmul(ps_t[0:H, 0:ncol], wt_j[:, c, :],
                                     xts_cur[c // 4][:, c % 4, 0:ncol],
                                     start=(c == 0), stop=(c == DCH - 1))

                # s^T + cvec (per-partition bias add, h on partitions)
                sT = st_pool.tile([H, 512], BF16, tag="st")
                nc.vector.tensor_scalar(sT[:, 0:ncol], ps_t[0:H, 0:ncol],
                                        cvec_c[:], None,
                                        mybir.AluOpType.add)

                # first-group cast (DVE, early) + first transpose half of
                # the next supergroup
                if si + 1 < len(SGS):
                    emit_cast(sg_groups(si + 1)[0], nc.vector)
                    state_next = {}
                    emit_xp_half(si + 1, state_next, 0)
                    xts_next = emit_copies_half(si + 1, state_next, 0, None)

                # back to natural layout [128, nk, 8] (tiny 8-row transposes)
                pu_t = pu_pool.tile([128, 4 * H], BF16, tag="pu")
                for j in range(nk):
                    nc.tensor.matmul(pu_t[:, j * H:(j + 1) * H],
                                     sT[:, j * 128:(j + 1) * 128],
                                     id128b[:H, :H],
                                     is_transpose=True, start=True,
                                     stop=True)

                # u = exp(leaky(s)) = max(exp(s), exp(0.01 s))
                e1 = e_pool.tile([128, 4 * H], F32, tag="e1")
                nc.scalar.activation(e1[:, 0:nk * H], pu_t[:, 0:nk * H],
                                     mybir.ActivationFunctionType.Exp)
                e2 = e_pool.tile([128, 4 * H], F32, tag="e2")
                nc.scalar.activation(e2[:, 0:nk * H], pu_t[:, 0:nk * H],
                                     mybir.ActivationFunctionType.Exp,
                                     scale=0.01)
                u_sg = u_all[:, k0:k1, :]
                nc.vector.tensor_tensor(
                    u_sg.rearrange("p a b -> p (a b)"),
                    e1[:, 0:nk * H], e2[:, 0:nk * H],
                    mybir.AluOpType.max)

                # late cast of the next supergroup's second group on ACT
                # (AFTER the exps: the exps are latency-critical, the cast
                # eats the remaining slack before xp-B needs it)
                if si + 1 < len(SGS):
                    gs = sg_groups(si + 1)
                    if len(gs) > 1:
                        emit_cast(gs[1], nc.scalar)
                    emit_xp_half(si + 1, state_next, 1)
                    xts_next = emit_copies_half(si + 1, state_next, 1,
                                                xts_next)

                # HO[h, d] += u^T X for this supergroup
                for j in range(nk):
                    k = k0 + j
                    g, j2 = k // KPD, k % KPD
                    for half, ho in ((0, ho0), (1, ho1)):
                        nc.tensor.matmul(
                            ho[:], u_all[:, k, :],
                            xb_tiles[g][:, j2,
                                        half * 512:(half + 1) * 512],
                            start=(k == 0), stop=(k == KCH - 1))

                if si + 1 < len(SGS):
                    xts_cur = xts_next

            # ---- ship HO partials + raw u (host reduces Z) ----
            ho_sb = small.tile([H, D], F32)
            nc.vector.tensor_copy(ho_sb[:, 0:512], ho0[:])
            nc.scalar.copy(ho_sb[:, 512:1024], ho1[:])

            nc.sync.dma_start(out=ho_out[:], in_=ho_sb[:])
            u_f32 = small.tile([128, KCH * H], F32)
            nc.vector.tensor_copy(
                u_f32[:], u_all[:].rearrange("p a b -> p (a b)"))
            nc.sync.dma_start(out=z_out[:], in_=u_f32[:])

    nc.compile()
    return nc


_CACHE = {}


def _get_program():
    if "nc" not in _CACHE:
        _CACHE["nc"] = _build()
    return _CACHE["nc"]


def _in_maps(final_result, W, b):
    final_result = np.ascontiguousarray(final_result, dtype=np.float32)
    W = np.ascontiguousarray(W, dtype=np.float32)
    b = np.ascontiguousarray(b, dtype=np.float32)
    x0 = np.ascontiguousarray(final_result[0]).reshape(H, 128)
    wbx = np.concatenate([W, b.reshape(H, 1), x0], axis=1)
    wbx = np.ascontiguousarray(wbx, dtype=np.float32)
    return [
        {
            "x": final_result[c * NSHARD:(c + 1) * NSHARD],
            "wbx": wbx,
        }
        for c in range(NCORES)
    ]


def _finalize(ho_sum, z_sum):
    r = (ho_sum / (H * z_sum[:, None])).sum(axis=0, dtype=np.float32)
    return np.maximum(r, np.float32(0)).astype(np.float32)


def kernel(final_result, W, b):
    nc = _get_program()
    res = run_bass_kernel_spmd(nc, _in_maps(final_result, W, b),
                               list(range(NCORES)))
    ho_sum = np.zeros((H, D), dtype=np.float32)
    z_sum = np.zeros((H,), dtype=np.float32)
    for c in range(NCORES):
        ho_sum += np.asarray(res.results[c]["ho"], dtype=np.float32)
        u = np.asarray(res.results[c]["z"], dtype=np.float32)
        z_sum += u.reshape(128 * KCH, H).sum(axis=0, dtype=np.float32)
    return _finalize(ho_sum, z_sum)


if __name__ == "__main__":
    rng = np.random.default_rng(0)
    x = rng.standard_normal((N, D), dtype=np.float32)
    W = (rng.standard_normal((H, 2 * D)) * 0.05).astype(np.float32)
    b = (rng.standard_normal(H) * 0.05).astype(np.float32)
    out = kernel(final_result=x, W=W, b=b)

    # reference check
    Wi, Wj = W[:, :D], W[:, D:]
    scores = x[0] @ Wi.T + x @ Wj.T + b
    scores = np.where(scores >= 0, scores, 0.01 * scores)
    scores -= scores.max(axis=0, keepdims=True)
    p = np.exp(scores)
    p /= p.sum(axis=0, keepdims=True)
    ref = np.maximum((p.T @ x).sum(axis=0) / H, 0)
    err = np.abs(out - ref).max() / np.abs(ref).max()
    print("kernel out:", out.shape, "rel err vs local ref:", err)


# revision 27
# speedup vs baseline: 1.0510x; 1.0510x over previous
"""Trainium2 Bass kernel for GAT-style single-query attention.

Reference computation (N=16384, D=1024, H=8):
    scores[n,h] = leaky_relu(x0 @ Wi[h] + x[n] @ Wj[h] + b[h], 0.01)
    probs       = softmax(scores, axis=n)  (per head)
    out[d]      = relu(mean_h(sum_n probs[n,h] * x[n,d]))

Strategy (v3): shard rows (N) across 8 cores.  Each core:
  - one packed DMA for the small inputs (W|b|x0 concatenated host-side),
    then 8 X-group DMAs (2 k-chunks of 128 rows each) issued immediately,
  - each X group is cast f32 -> bf16 once (DVE / GpSimd alternating);
    every PE instruction downstream runs in bf16 (1 cycle/row -- measured
    f32r streams at ~2 cycles/row on HW, so bf16 halves matmul time),
  - X^T via PE transposes of the bf16 tiles into 1-bank PSUM tiles
    ([128,1024] bf16), one 2x-speed DVE copy per c-pair,
  - scores^T [8, <=512] per supergroup; the per-head constant
    cvec[h] = x0 @ Wi[h] + b[h] is folded into the PSUM->SBUF copy as a
    per-partition tensor_scalar add (h is the partition dim there),
  - scores^T transposed back to natural [128, k, 8] layout on the PE
    (tiny 8-row transposes), so exp/leaky runs on few-free-element tiles
    (DVE/ACT cost ~ free size, partitions are parallel),
  - u = exp(leaky(s)) = max(exp(s), exp(0.01 s)) (exp monotone; scores
    are in [-9, 8] for this distribution, no max-subtraction needed),
  - HO[h, d] += u^T X on the PE (u natural stationary, X natural moving),
  - supergroups sized [4,4,4,2,2] k-chunks so the post-DMA tail is short,
  - ships HO partials [8, 1024] plus the raw u tile [128, 128] (f32);
    the host reduces Z_h and finishes relu(mean_h HO_h / Z_h).
"""

import sys

sys.path.insert(0, "/opt/trn_rl_repo")

import numpy as np

import concourse.bacc as bacc
import concourse.tile as tile
from concourse import mybir
from concourse import masks
from concourse.bass_utils import run_bass_kernel_spmd

N, D, H = 16384, 1024, 8
NCORES = 8
NSHARD = N // NCORES          # 2048 rows per core
KCH = NSHARD // 128           # 16 n-chunks of 128 rows
DCH = 8                       # d-chunks of 128 cols
NDMA = 8                      # DMA groups (2 k-chunks each)
KPD = KCH // NDMA             # k-chunks per DMA group (2)
SGS = [(0, 4), (4, 8), (8, 12), (12, 14), (14, 16)]  # supergroup k-ranges
F32 = mybir.dt.float32
BF16 = mybir.dt.bfloat16
WBX_W = 2 * D + 1 + 128       # W[8,2048] | b[8,1] | x0 as [8,128]
N_WARMUP = 3                  # lean f32 warm-up matmuls


def _build():
    nc = bacc.Bacc("TRN2", target_bir_lowering=False, debug=False,
                   num_devices=NCORES)
    x_in = nc.dram_tensor("x", [NSHARD, D], F32, kind="ExternalInput").ap()
    wbx_in = nc.dram_tensor("wbx", [H, WBX_W], F32,
                            kind="ExternalInput").ap()
    ho_out = nc.dram_tensor("ho", [H, D], F32, kind="ExternalOutput").ap()
    z_out = nc.dram_tensor("z", [128, KCH * H], F32,
                           kind="ExternalOutput").ap()

    with tile.TileContext(nc) as tc:
        with (
            tc.tile_pool(name="consts", bufs=1) as consts,
            tc.tile_pool(name="small", bufs=1) as small,
            tc.tile_pool(name="xn", bufs=1) as xn_pool,
            tc.tile_pool(name="xb", bufs=1) as xb_pool,
            tc.tile_pool(name="xt", bufs=1) as xt_pool,
            tc.tile_pool(name="st", bufs=2) as st_pool,
            tc.tile_pool(name="ee", bufs=2) as e_pool,
            tc.tile_pool(name="pt", bufs=4, space="PSUM") as pt_pool,
            tc.tile_pool(name="ps", bufs=1, space="PSUM") as ps_pool,
            tc.tile_pool(name="pho", bufs=1, space="PSUM") as pho_pool,
            tc.tile_pool(name="pu", bufs=1, space="PSUM") as pu_pool,
        ):
            # ---- warm-up operands: the very first DVE instructions ----
            wa = consts.tile([128, H], F32)
            nc.vector.memset(wa[:], 0.001)
            wb = consts.tile([128, 512], F32)
            nc.vector.memset(wb[:], 0.001)

            # identity on gpsimd (parallel with the DVE memsets)
            id128 = consts.tile([128, 128], F32)
            masks.make_identity(nc, id128[:])
            id128b = consts.tile([128, 128], BF16)
            nc.vector.tensor_copy(id128b[:], id128[:])

            # preload the ACT exp table during the DMA wait
            act_dummy = consts.tile([1, H], F32)
            nc.scalar.activation(act_dummy[:], wa[0:1, :],
                                 mybir.ActivationFunctionType.Exp)

            # ---- DMA: X group 0 first, then the small input, then the
            # remaining X groups (queues drain in trigger order) ----
            wbx_sb = small.tile([H, WBX_W], F32)
            x_view = x_in.rearrange("(p k) d -> p k d", k=KCH)
            xn_tiles = [xn_pool.tile([128, KPD, D], F32, tag=f"xn{g}",
                                     name=f"xn{g}")
                        for g in range(NDMA)]
            nc.sync.dma_start(out=wbx_sb[:], in_=wbx_in[:])
            for g in range(NDMA):
                nc.sync.dma_start(
                    out=xn_tiles[g][:],
                    in_=x_view[:, g * KPD:(g + 1) * KPD, :])

            # ---- PE warm-up: 3 f32 matmuls on alternating banks ----
            ho0 = pho_pool.tile([H, 512], F32, tag="ho0")
            ho1 = pho_pool.tile([H, 512], F32, tag="ho1")
            for i in range(N_WARMUP):
                nc.tensor.matmul((ho0 if i % 2 == 0 else ho1)[:],
                                 wa[:], wb[:], start=True, stop=True)

            # bf16 copy of the packed small input (for W/x0 transposes)
            wbx_b = small.tile([H, WBX_W], BF16)
            nc.vector.tensor_copy(wbx_b[:], wbx_sb[:])


            # bf16 casts of the X groups (cast engine interleaved so the
            # PE never waits: first group of each sg on DVE early, second
            # on ACT).  sg0's casts go out NOW -- before the W copies,
            # which are gated on PE transposes and would delay them.
            xb_tiles = {}

            def emit_cast(g, eng):
                # split each group cast DVE/ACT half-and-half: the two
                # halves run in parallel, halving the cast latency on the
                # DMA->cast->transpose critical chain
                if g in xb_tiles:
                    return
                xb = xb_pool.tile([128, KPD, D], BF16, tag=f"xb{g}")
                xb_tiles[g] = xb
                nc.vector.tensor_copy(xb[:, 0, :], xn_tiles[g][:, 0, :])
                nc.scalar.copy(xb[:, 1, :], xn_tiles[g][:, 1, :])

            def sg_groups(si):
                k0, k1 = SGS[si]
                return sorted({k // KPD for k in range(k0, k1)})

            gs0 = sg_groups(0)
            emit_cast(gs0[0], nc.vector)
            if len(gs0) > 1:
                emit_cast(gs0[1], nc.scalar)

            # sg0 casts early in DVE/ACT program order (W copies below
            # are gated on PE transposes and would delay them by ~3us)

            # ---- W^T chunks via bf16 PE transposes ----
            wt_j = small.tile([128, DCH, H], BF16)
            wt_i = small.tile([128, DCH, H], BF16)
            for half, dst in ((1, wt_j), (0, wt_i)):
                pw = pt_pool.tile([128, 2 * 512], BF16, tag="pt")
                for c in range(DCH):
                    nc.tensor.matmul(
                        pw[:, c * H:(c + 1) * H],
                        wbx_b[:, (half * DCH + c) * 128:
                              (half * DCH + c + 1) * 128],
                        id128b[:H, :H],
                        is_transpose=True, start=True, stop=True)
                nc.vector.tensor_copy(dst[:], pw[:, 0:DCH * H])

            # x0^T: [8, 128] slice -> [128, 8]
            px = pt_pool.tile([128, 2 * 512], BF16, tag="pt")
            nc.tensor.matmul(px[:, 0:H], wbx_b[:, 2 * D + 1:],
                             id128b[:H, :H],
                             is_transpose=True, start=True, stop=True)
            x0t = small.tile([128, H], BF16)
            nc.vector.tensor_copy(x0t[:], px[:, 0:H])

            # cvec[h] = x0 @ Wi[h] + b[h], as an [8, 1] column
            pc = ps_pool.tile([64, 512], F32, tag="ps")
            for c in range(DCH):
                nc.tensor.matmul(pc[0:H, 0:1], wt_i[:, c, :],
                                 x0t[:, c:c + 1],
                                 start=(c == 0), stop=(c == DCH - 1))
            cvec_c = small.tile([H, 1], F32)
            nc.vector.tensor_tensor(cvec_c[:], pc[0:H, 0:1],
                                    wbx_sb[:, 2 * D:2 * D + 1],
                                    mybir.AluOpType.add)

            # u partial sums, natural layout: [128, k, 8] across all k
            u_all = small.tile([128, KCH, H], BF16)


            def emit_xp_half(si, state, which):
                """Transposes for supergroup si, split by DMA group.
                which=0: first group's j-chunks; which=1: the rest."""
                k0, k1 = SGS[si]
                nk = k1 - k0
                gs = sg_groups(si)
                if which == 1 and len(gs) == 1:
                    return
                gsel = gs[which]
                if not state:
                    for cp in range(DCH // 2):
                        state[cp] = pt_pool.tile([128, 2 * 512], BF16,
                                                 tag="pt",
                                                 name=f"ptt{si}_{cp}")
                for cp in range(DCH // 2):
                    ptt = state[cp]
                    for j in range(nk):
                        k = k0 + j
                        g, j2 = k // KPD, k % KPD
                        if g != gsel:
                            continue
                        for ci in range(2):
                            c = cp * 2 + ci
                            nc.tensor.matmul(
                                ptt[:, ci * 512 + j * 128:
                                    ci * 512 + (j + 1) * 128],
                                xb_tiles[g][:, j2, c * 128:(c + 1) * 128],
                                id128b[:],
                                is_transpose=True, start=True, stop=True)

            def emit_copies(si, state):
                k0, k1 = SGS[si]
                nk = k1 - k0
                xts = []
                for cp in range(DCH // 2):
                    ptt = state[cp]
                    xt = xt_pool.tile([128, 2, 512], BF16,
                                      tag=f"xt{si}_{cp}",
                                      name=f"xt{si}_{cp}")
                    ptt3 = ptt[:].rearrange("p (a b) -> p a b", b=512)
                    nc.vector.tensor_copy(
                        xt[:, :, 0:nk * 128], ptt3[:, :, 0:nk * 128])
                    xts.append(xt)
                return xts

            # prologue: transposes + copies for sg 0 (casts already out)
            state0 = {}
            emit_xp_half(0, state0, 0)
            emit_xp_half(0, state0, 1)
            xts_cur = emit_copies(0, state0)
            state_next = None
            xts_next = None
            for si, (k0, k1) in enumerate(SGS):
                nk = k1 - k0
                ncol = nk * 128
                # scores^T [8, ncol] for this supergroup
                ps_t = ps_pool.tile([64, 512], F32, tag="ps")
                for c in range(DCH):
                    nc.tensor.matmul(ps_t[0:H, 0:ncol], wt_j[:, c, :],
                                     xts_cur[c // 2][:, c % 2, 0:ncol],
                                     start=(c == 0), stop=(c == DCH - 1))

                # s^T + cvec (per-partition bias add, h on partitions)
                sT = st_pool.tile([H, 512], BF16, tag="st")
                nc.vector.tensor_scalar(sT[:, 0:ncol], ps_t[0:H, 0:ncol],
                                        cvec_c[:], None,
                                        mybir.AluOpType.add)

                # first-group cast (DVE, early) + first transpose half of
                # the next supergroup
                if si + 1 < len(SGS):
                    emit_cast(sg_groups(si + 1)[0], nc.vector)
                    state_next = {}
                    emit_xp_half(si + 1, state_next, 0)

                # back to natural layout [128, nk, 8] (tiny 8-row transposes)
                pu_t = pu_pool.tile([128, 4 * H], BF16, tag="pu")
                for j in range(nk):
                    nc.tensor.matmul(pu_t[:, j * H:(j + 1) * H],
                                     sT[:, j * 128:(j + 1) * 128],
                                     id128b[:H, :H],
                                     is_transpose=True, start=True,
                                     stop=True)

                # u = exp(leaky(s)) = max(exp(s), exp(0.01 s))
                e1 = e_pool.tile([128, 4 * H], F32, tag="e1")
                nc.scalar.activation(e1[:, 0:nk * H], pu_t[:, 0:nk * H],
                                     mybir.ActivationFunctionType.Exp)
                e2 = e_pool.tile([128, 4 * H], F32, tag="e2")
                nc.scalar.activation(e2[:, 0:nk * H], pu_t[:, 0:nk * H],
                                     mybir.ActivationFunctionType.Exp,
                                     scale=0.01)
                u_sg = u_all[:, k0:k1, :]
                nc.vector.tensor_tensor(
                    u_sg.rearrange("p a b -> p (a b)"),
                    e1[:, 0:nk * H], e2[:, 0:nk * H],
                    mybir.AluOpType.max)

                # late cast of the next supergroup's second group on ACT
                # (AFTER the exps: the exps are latency-critical, the cast
                # eats the remaining slack before xp-B needs it)
                if si + 1 < len(SGS):
                    gs = sg_groups(si + 1)
                    if len(gs) > 1:
                        emit_cast(gs[1], nc.scalar)
                    emit_xp_half(si + 1, state_next, 1)

                # HO[h, d] += u^T X for this supergroup
                for j in range(nk):
                    k = k0 + j
                    g, j2 = k // KPD, k % KPD
                    for half, ho in ((0, ho0), (1, ho1)):
                        nc.tensor.matmul(
                            ho[:], u_all[:, k, :],
                            xb_tiles[g][:, j2,
                                        half * 512:(half + 1) * 512],
                            start=(k == 0), stop=(k == KCH - 1))

                if si + 1 < len(SGS):
                    xts_cur = emit_copies(si + 1, state_next)

            # ---- ship HO partials + raw u (host reduces Z) ----
            ho_sb = small.tile([H, D], F32)
            nc.vector.tensor_copy(ho_sb[:, 0:512], ho0[:])
            nc.scalar.copy(ho_sb[:, 512:1024], ho1[:])

            nc.sync.dma_start(out=ho_out[:], in_=ho_sb[:])
            u_f32 = small.tile([128, KCH * H], F32)
            nc.vector.tensor_copy(
                u_f32[:], u_all[:].rearrange("p a b -> p (a b)"))
            nc.sync.dma_start(out=z_out[:], in_=u_f32[:])

    nc.compile()
    return nc


_CACHE = {}


def _get_program():
    if "nc" not in _CACHE:
        _CACHE["nc"] = _build()
    return _CACHE["nc"]


def _in_maps(final_result, W, b):
    final_result = np.ascontiguousarray(final_result, dtype=np.float32)
    W = np.ascontiguousarray(W, dtype=np.float32)
    b = np.ascontiguousarray(b, dtype=np.float32)
    x0 = np.ascontiguousarray(final_result[0]).reshape(H, 128)
    wbx = np.concatenate([W, b.reshape(H, 1), x0], axis=1)
    wbx = np.ascontiguousarray(wbx, dtype=np.float32)
    return [
        {
            "x": final_result[c * NSHARD:(c + 1) * NSHARD],
            "wbx": wbx,
        }
        for c in range(NCORES)
    ]


def _finalize(ho_sum, z_sum):
    r = (ho_sum / (H * z_sum[:, None])).sum(axis=0, dtype=np.float32)
    return np.maximum(r, np.float32(0)).astype(np.float32)


def kernel(final_result, W, b):
    nc = _get_program()
    res = run_bass_kernel_spmd(nc, _in_maps(final_result, W, b),
                               list(range(NCORES)))
    ho_sum = np.zeros((H, D), dtype=np.float32)
    z_sum = np.zeros((H,), dtype=np.float32)
    for c in range(NCORES):
        ho_sum += np.asarray(res.results[c]["ho"], dtype=np.float32)
        u = np.asarray(res.results[c]["z"], dtype=np.float32)
        z_sum += u.reshape(128 * KCH, H).sum(axis=0, dtype=np.float32)
    return _finalize(ho_sum, z_sum)


if __name__ == "__main__":
    rng = np.random.default_rng(0)
    x = rng.standard_normal((N, D), dtype=np.float32)
    W = (rng.standard_normal((H, 2 * D)) * 0.05).astype(np.float32)
    b = (rng.standard_normal(H) * 0.05).astype(np.float32)
    out = kernel(final_result=x, W=W, b=b)

    # reference check
    Wi, Wj = W[:, :D], W[:, D:]
    scores = x[0] @ Wi.T + x @ Wj.T + b
    scores = np.where(scores >= 0, scores, 0.01 * scores)
    scores -= scores.max(axis=0, keepdims=True)
    p = np.exp(scores)
    p /= p.sum(axis=0, keepdims=True)
    ref = np.maximum((p.T @ x).sum(axis=0) / H, 0)
    err = np.abs(out - ref).max() / np.abs(ref).max()
    print("kernel out:", out.shape, "rel err vs local ref:", err)
